# revision 11
# baseline (speedup 1.0000x reference)
"""Trainium2 Bass kernel for DisturbanceRegressionLoss2Heads.

Reference computation (per batch element b, per pixel (h, w)):
  y0 = out[b, 0]  (Y=30 time steps)   y1 = out[b, 1]
  diff = [-7, 0, y0[2]-y0[1], ..., y0[28]-y0[27], 0]
  d = argmin(diff)  (first min)
  piecewise OLS fit of y0 over t<d (x=t) and t>=d (x=t-d), slopes clipped to
  [0,2] in the fitted line, intercepts clipped to [0,100]
  loss = mean over everything of (fitted - y1)^2

Strategy: pure data parallel over the batch (8 cores, one batch element each).
Per core, pixels are tiled [128 partitions x F pixels/partition] and the
30-step time axis lives in the free dimension; chunks are double-buffered so
DMA and the three compute engines overlap.  The argmin uses a compressed
28-slot diff sequence [-7, diff_2..diff_28] (t=1/t=29 entries are 0 and can
never beat -7): ONE running-min tensor_tensor_scan with multiply-by-zero
per-pixel state resets, then mask(t<d) = (running_min > final_min) via a
broadcast view of the scan's last slot -- exact first-min semantics matching
jnp.argmin, with d recovered as sum(mask).  Segment sums come from masked
products + innermost-axis reduces; a per-pixel OLS epilogue forms clipped
slopes/intercepts; the piecewise-fitted curve is assembled with broadcast
APs + copy_predicated and the squared residual is accumulated per-partition
by the scalar engine's activation(Square, accum_out).
Each core writes 128 x NCHUNK partial sums; the host sums them in float64.

Built with bacc.Bacc: its generate_event_semaphores pass splits multi-wait
sync (TRN2 allows at most one wait per instruction) — raw bass.Bass kernels
of this shape fail walrus codegen or corrupt sync and hang the device.
"""

import numpy as np

import concourse.bacc as bacc
import concourse.tile as tile
from concourse import mybir
from concourse.bass_utils import run_bass_kernel_spmd

F32 = mybir.dt.float32
BF16 = mybir.dt.bfloat16
AX = mybir.AxisListType
OP = mybir.AluOpType
AF = mybir.ActivationFunctionType

B = 8
Y = 30
H = 256
W = 256
NPIX = H * W          # 65536 pixels per core
P = 128               # SBUF partitions
FP = NPIX // P        # 512 pixels per partition, whole core
F = 128               # pixels per partition per chunk
CHUNK = P * F
NCHUNK = FP // F      # 4
DIST = 7.0
MAXI = 100.0

# engine assignment config (tuned against the TimelineSim cost model)
CFG = {"diff_g": False, "pbt_g": True, "y0t_g": True, "fb_g": True,
       "neq_g": False, "fa_g": False, "r_g": True,
       "io_bufs": 2, "work_bufs": 2, "sm_bufs": 2, "fa_tag": "dpt"}


def _emit_chunk(nc, pools, c, y0d, y1d, z28, tb, partial):
    """One chunk: argmin, masked sums, OLS epilogue, fitted curve, residual."""
    io, work, sm = pools
    cs = c * CHUNK

    y0a = io.tile([P, F, Y], F32, tag="y0a")
    y1a = io.tile([P, F, Y], F32, tag="y1a")
    nc.sync.dma_start(out=y0a[:],
                      in_=y0d[cs:cs + CHUNK, :].rearrange("(p f) y -> p f y",
                                                          p=P))
    nc.sync.dma_start(out=y1a[:],
                      in_=y1d[cs:cs + CHUNK, :].rearrange("(p f) y -> p f y",
                                                          p=P))
    y0pt = y0a[:]                               # [P, F, Y] unit-stride
    y1pt = y1a[:]

    def b3(small):  # broadcast a [P, F] per-pixel tile along t
        return small[:][:, :, None].broadcast_to([P, F, Y])

    def b28(small):  # broadcast a [P, F] per-pixel tile along 28 slots
        return small[:][:, :, None].broadcast_to([P, F, 28])

    # ---- compressed diff sequence, 28 slots per pixel:
    #      [-7, diff_2, ..., diff_28]  (t=1 and t=29 entries are 0 in the
    #      reference and can never be the argmin since -7 < 0)
    dpt = work.tile([P, F, 28], F32, tag="dpt")
    deng = nc.gpsimd if CFG["diff_g"] else nc.vector
    deng.tensor_tensor(
        out=dpt[:, :, 1:28], in0=y0pt[:, :, 2:29], in1=y0pt[:, :, 1:28],
        op=OP.subtract)
    deng.memset(dpt[:, :, 0:1], -DIST)

    # ---- running min over slots (z28 resets state at each pixel start);
    #      maskB[t] = [t < d] == [runmin(t) > min] exactly (first-min
    #      semantics match jnp.argmin: before the first minimum the running
    #      min is strictly above the global min)
    rmf = work.tile([P, F, 28], F32, tag="M")
    nc.vector.tensor_tensor_scan(
        out=rmf[:].rearrange("p f y -> p (f y)"),
        data0=z28[:].rearrange("p f y -> p (f y)"),
        data1=dpt[:].rearrange("p f y -> p (f y)"),
        initial=0.0, op0=OP.mult, op1=OP.min)
    maskB = work.tile([P, F, Y], F32, tag="maskB")
    # slot s corresponds to t = s+1; cols 0 and 1 share slot 0's value
    # ([d >= 2] equals [0 < d] because d is never 1); the global min is
    # read as a broadcast view of the last running-min slot
    mfin = rmf[:, :, 27:28]
    nc.vector.tensor_tensor(out=maskB[:, :, 2:29], in0=rmf[:, :, 1:28],
                            in1=mfin.broadcast_to([P, F, 27]), op=OP.is_gt)
    nc.vector.tensor_tensor(out=maskB[:, :, 0:2],
                            in0=rmf[:, :, 0:1].broadcast_to([P, F, 2]),
                            in1=mfin.broadcast_to([P, F, 2]), op=OP.is_gt)
    nc.vector.memset(maskB[:, :, 29:30], 0.0)
    d = sm.tile([P, F], F32, tag="d")
    nc.vector.tensor_reduce(out=d[:], in_=maskB[:], axis=AX.X, op=OP.add)

    pb = work.tile([P, F, Y], F32, tag="dpt")     # reuse dpt slot group
    nc.vector.tensor_tensor(out=pb[:], in0=maskB[:], in1=y0pt, op=OP.mult)
    syb = sm.tile([P, F], F32, tag="syb")
    nc.vector.tensor_reduce(out=syb[:], in_=pb[:], axis=AX.X, op=OP.add)

    # t-weighted masked sum: multiply pb by t in place, then reduce
    peng = nc.gpsimd if CFG["pbt_g"] else nc.vector
    peng.tensor_tensor(out=pb[:], in0=pb[:], in1=tb, op=OP.mult)
    styb = sm.tile([P, F], F32, tag="styb")
    nc.vector.tensor_reduce(out=styb[:], in_=pb[:], axis=AX.X, op=OP.add)

    # totals: reduce y0, then scale y0 by t in place (its last use), reduce
    ty = sm.tile([P, F], F32, tag="ty")
    nc.vector.tensor_reduce(out=ty[:], in_=y0pt, axis=AX.X, op=OP.add)
    yeng = nc.gpsimd if CFG["y0t_g"] else nc.vector
    yeng.tensor_tensor(out=y0pt, in0=y0pt, in1=tb, op=OP.mult)
    tty = sm.tile([P, F], F32, tag="tty")
    nc.vector.tensor_reduce(out=tty[:], in_=y0pt, axis=AX.X, op=OP.add)

    # ---- per-pixel regression epilogue ([P, F] smalls)
    def tt(name, a, bb, op):
        t = sm.tile([P, F], F32, tag=name)
        nc.vector.tensor_tensor(out=t[:], in0=a[:], in1=bb[:], op=op)
        return t

    na = sm.tile([P, F], F32, tag="na")           # 30 - d
    nc.scalar.activation(out=na[:], in_=d[:], func=AF.Copy, bias=float(Y),
                         scale=-1.0)
    sya = tt("sya", ty, syb, OP.subtract)         # sum y, t>=d
    t0 = tt("t0", tty, styb, OP.subtract)         # sum t*y, t>=d
    t1 = tt("t1", d, sya, OP.mult)
    nc.vector.tensor_tensor(out=t0[:], in0=t0[:], in1=t1[:], op=OP.subtract)
    sxya = t0                                     # sum (t-d)*y, t>=d

    nbs = sm.tile([P, F], F32, tag="nbs")
    nc.vector.tensor_scalar(out=nbs[:], in0=d[:], scalar1=1.0, scalar2=None,
                            op0=OP.max)
    nc.vector.reciprocal(out=nbs[:], in_=nbs[:])
    ra = sm.tile([P, F], F32, tag="ra")
    nc.vector.reciprocal(out=ra[:], in_=na[:])

    myb = tt("myb", syb, nbs, OP.mult)            # mean y before
    mya = tt("mya", sya, ra, OP.mult)             # mean y after
    mxb = sm.tile([P, F], F32, tag="mxb")         # (d-1)/2
    nc.scalar.activation(out=mxb[:], in_=d[:], func=AF.Copy, bias=-0.5,
                         scale=0.5)
    mxa = sm.tile([P, F], F32, tag="mxa")         # (na-1)/2
    nc.scalar.activation(out=mxa[:], in_=na[:], func=AF.Copy, bias=-0.5,
                         scale=0.5)

    covb = tt("covb", mxb, syb, OP.mult)          # mxb*syb, then styb - that
    nc.vector.tensor_tensor(out=covb[:], in0=styb[:], in1=covb[:],
                            op=OP.subtract)
    cova = tt("cova", mxa, sya, OP.mult)
    nc.vector.tensor_tensor(out=cova[:], in0=sxya[:], in1=cova[:],
                            op=OP.subtract)

    # var*12 = n*(n^2-1); slope = cov / max(var, 1) gated on var > 0
    vb12 = tt("vb12", d, d, OP.mult)
    nc.vector.scalar_tensor_tensor(out=vb12[:], in0=vb12[:], scalar=1.0,
                                   in1=d[:], op0=OP.subtract, op1=OP.mult)
    mvb = sm.tile([P, F], F32, tag="mvb")
    nc.vector.tensor_scalar(out=mvb[:], in0=vb12[:], scalar1=1.0 / 12.0,
                            scalar2=1.0, op0=OP.mult, op1=OP.max)
    nc.vector.reciprocal(out=mvb[:], in_=mvb[:])
    slb = tt("slb", covb, mvb, OP.mult)
    nc.vector.tensor_scalar(out=vb12[:], in0=vb12[:], scalar1=0.0, scalar2=None,
                            op0=OP.is_gt)        # gate, reuses vb12
    nc.vector.tensor_tensor(out=slb[:], in0=slb[:], in1=vb12[:], op=OP.mult)

    va12 = tt("va12", na, na, OP.mult)
    nc.vector.scalar_tensor_tensor(out=va12[:], in0=va12[:], scalar=1.0,
                                   in1=na[:], op0=OP.subtract, op1=OP.mult)
    nc.vector.tensor_scalar(out=va12[:], in0=va12[:], scalar1=1.0 / 12.0,
                            scalar2=1.0, op0=OP.mult, op1=OP.max)
    nc.vector.reciprocal(out=va12[:], in_=va12[:])
    sla = tt("sla", cova, va12, OP.mult)          # slope after (na>=2 always)

    # intercepts use the *unclipped* slope; fitted lines use clipped slopes
    ibv = tt("ibv", slb, mxb, OP.mult)
    nc.vector.tensor_tensor(out=ibv[:], in0=myb[:], in1=ibv[:], op=OP.subtract)
    nc.vector.tensor_scalar(out=ibv[:], in0=ibv[:], scalar1=0.0, scalar2=MAXI,
                            op0=OP.max, op1=OP.min)
    iav = tt("iav", sla, mxa, OP.mult)
    nc.vector.tensor_tensor(out=iav[:], in0=mya[:], in1=iav[:], op=OP.subtract)
    nc.vector.tensor_scalar(out=iav[:], in0=iav[:], scalar1=0.0, scalar2=MAXI,
                            op0=OP.max, op1=OP.min)
    sbc = sm.tile([P, F], F32, tag="sbc")
    nc.vector.tensor_scalar(out=sbc[:], in0=slb[:], scalar1=0.0, scalar2=2.0,
                            op0=OP.max, op1=OP.min)
    sac = sm.tile([P, F], F32, tag="sac")
    nc.vector.tensor_scalar(out=sac[:], in0=sla[:], scalar1=0.0, scalar2=2.0,
                            op0=OP.max, op1=OP.min)
    ia2f = tt("ia2f", sac, d, OP.mult)            # ia - sac*d
    nc.vector.tensor_tensor(out=ia2f[:], in0=iav[:], in1=ia2f[:],
                            op=OP.subtract)

    # ---- fitted curve: fa = sac*t + ia2f, overwritten with fb = sbc*t + ibv
    # where t < d; then residual vs y1, square + accumulate on scalar engine.
    fa = work.tile([P, F, Y], F32, tag=CFG["fa_tag"])
    aeng = nc.gpsimd if CFG["fa_g"] else nc.vector
    aeng.tensor_tensor(out=fa[:], in0=b3(sac), in1=tb, op=OP.mult)
    aeng.tensor_tensor(out=fa[:], in0=fa[:], in1=b3(ia2f), op=OP.add)
    fb = work.tile([P, F, Y], F32, tag="M")
    feng = nc.gpsimd if CFG["fb_g"] else nc.vector
    feng.tensor_tensor(out=fb[:], in0=b3(sbc), in1=tb, op=OP.mult)
    feng.tensor_tensor(out=fb[:], in0=fb[:], in1=b3(ibv), op=OP.add)
    nc.vector.copy_predicated(out=fa[:], mask=maskB[:].bitcast(mybir.dt.int32),
                              data=fb[:])
    reng = nc.gpsimd if CFG["r_g"] else nc.vector
    reng.tensor_tensor(out=fa[:], in0=fa[:], in1=y1pt, op=OP.subtract)
    nc.scalar.activation(out=fa[:], in_=fa[:], func=AF.Square,
                         accum_out=partial[:, c:c + 1])


def build_core_program():
    """Build the per-core Bass program (same program on all 8 cores)."""
    from contextlib import ExitStack

    nc = bacc.Bacc(trn_type="TRN2")
    y0d = nc.dram_tensor("y0", [NPIX, Y], F32, kind="ExternalInput")
    y1d = nc.dram_tensor("y1", [NPIX, Y], F32, kind="ExternalInput")
    outd = nc.dram_tensor("partial", [P, NCHUNK], F32, kind="ExternalOutput")

    with tile.TileContext(nc) as tc, ExitStack() as ctx:
        singles = ctx.enter_context(tc.tile_pool(name="singles", bufs=1))
        io = ctx.enter_context(tc.tile_pool(name="io", bufs=CFG["io_bufs"]))
        work = ctx.enter_context(tc.tile_pool(name="work",
                                              bufs=CFG["work_bufs"]))
        sm = ctx.enter_context(tc.tile_pool(name="sm", bufs=CFG["sm_bufs"]))

        # constants: z28 (f32, 1 except 0 at slot 0 of each pixel's 28-slot
        # diff sequence; multiplying the scan state by 0 restarts the
        # running min at each pixel), t tile (bf16; values 0..29 are exact,
        # engines upconvert to fp32 internally)
        z28 = singles.tile([P, F, 28], F32)
        nc.vector.memset(z28[:], 1.0)
        nc.vector.memset(z28[:, :, 0:1], 0.0)
        trow_i = sm.tile([P, Y], mybir.dt.int32, tag="trow_i")
        nc.gpsimd.iota(trow_i[:], pattern=[[1, Y]], base=0, channel_multiplier=0)
        trow = sm.tile([P, Y], F32, tag="trow")
        nc.vector.tensor_copy(trow[:], trow_i[:])
        tvec = singles.tile([P, F, Y], BF16)
        nc.vector.tensor_copy(
            tvec[:], trow[:][:, None, :].broadcast_to([P, F, Y]))
        tb = tvec[:]
        partial = singles.tile([P, NCHUNK], F32)

        pools = (io, work, sm)
        for c in range(NCHUNK):
            _emit_chunk(nc, pools, c, y0d, y1d, z28, tb, partial)

        nc.sync.dma_start(out=outd[:, :], in_=partial[:])

    nc.finalize()   # Bacc: runs reg-alloc + the 1-wait sync-split lowering
    return nc


_NC = None


def _get_nc():
    global _NC
    if _NC is None:
        _NC = build_core_program()
    return _NC


def kernel(out, target=None, **_ignored):
    """Full-input entry point: shards batch over 8 cores, returns scalar loss."""
    out = np.ascontiguousarray(out, dtype=np.float32)
    assert out.shape == (B, 2, Y, H, W), out.shape
    nc = _get_nc()
    in_maps = [
        {
            "y0": np.ascontiguousarray(out[b, 0].reshape(Y, NPIX).T),
            "y1": np.ascontiguousarray(out[b, 1].reshape(Y, NPIX).T),
        }
        for b in range(B)
    ]
    res = run_bass_kernel_spmd(nc, in_maps, core_ids=list(range(B)))
    total = sum(r["partial"].astype(np.float64).sum() for r in res.results)
    loss = total / float(B * Y * NPIX)
    return np.float32(loss)

